# revision 19
# baseline (speedup 1.0000x reference)
"""BoxFilter (9x9 mean filter, reflect padding) Trainium2 Bass kernel.

Input x: [8, 3, 2048, 2048] f32, r=4. Output same shape.

Strategy (per NeuronCore; pure data parallel, batch b -> core b):
  - H-axis box sum via TensorEngine: banded-matrix matmuls (float32r) with
    reflection folded into edge band matrices and the 1/81 scale folded into
    the coefficients. Per 128-row output tile, accumulate contributions from
    the previous tile's last 4 rows, the current tile, and the next tile's
    first 4 rows into PSUM.
  - W-axis box sum via VectorEngine: reflect-pad the H-summed tile into a
    [128, 2057] SBUF tile (ScalarEngine copies from PSUM), run a prefix-sum
    along the free dim (tensor_tensor_scan), then one shifted subtract:
    out[w] = C[w+9] - C[w].
"""

import os
import sys

import numpy as np

for _p in ("/opt/trn_rl_repo", "/opt/pypackages"):
    if os.path.isdir(_p) and _p not in sys.path:
        sys.path.append(_p)

from contextlib import ExitStack

import concourse.bacc as bacc
import concourse.mybir as mybir
from concourse.tile import TileContext
from concourse.bass_utils import run_bass_kernel_spmd

R = 4
NORM = (2 * R + 1) ** 2  # 81
H = W = 2048
P = 128
NT = H // P  # 16 row tiles per image
NIMG = 3  # images per core (batch b -> core b, 3 channels each)
NCORES = 8
CH = 512  # psum chunk (one bank of f32)
F32 = mybir.dt.float32
F32R = mybir.dt.float32r
F16 = mybir.dt.float16
PW = W + 2 * R + 1  # padded width 2057


M_EDGE = P - R  # 124-row edge tiles
M_MID = P - 2 * R  # 120-row interior tiles
SIZES = [M_EDGE] + [M_MID] * 15 + [M_EDGE]  # 17 output tiles per image
OFFS = [0]
for _m in SIZES:
    OFFS.append(OFFS[-1] + _m)
assert OFFS[-1] == H
NTILE = len(SIZES)


def _band_blocks():
    """Banded H-axis operator blocks (lhsT, [K=128, M]) for the 17-tile scheme.

    Output tile t covers rows [OFFS[t], OFFS[t]+SIZES[t]); its input tile is
    the 128 rows [120t, 120t+128). Reflection at the image edges is folded
    into a_first / a_last; one K=128 matmul per output tile per psum chunk.
    """
    hop = np.zeros((H, H), np.float32)
    inv = np.float32(1.0) / np.float32(NORM)
    for i in range(H):
        for dh in range(-R, R + 1):
            g = i + dh
            if g < 0:
                g = -g
            elif g > H - 1:
                g = 2 * (H - 1) - g
            hop[i, g] += inv
    a_first = np.ascontiguousarray(hop[0:M_EDGE, 0:P].T)
    a_mid = np.ascontiguousarray(hop[M_EDGE : M_EDGE + M_MID, M_MID : M_MID + P].T)
    a_last = np.ascontiguousarray(hop[H - M_EDGE :, H - P :].T)
    return a_first, a_mid, a_last


def _build_nc():
    nc = bacc.Bacc("TRN2", target_bir_lowering=False, debug=False, num_devices=1)
    x = nc.declare_dram_parameter("x", [NIMG, H, W], F32R, isOutput=False)
    a_first = nc.declare_dram_parameter("a_first", [P, M_EDGE], F32R, isOutput=False)
    a_mid = nc.declare_dram_parameter("a_mid", [P, M_MID], F32R, isOutput=False)
    a_last = nc.declare_dram_parameter("a_last", [P, M_EDGE], F32R, isOutput=False)
    y = nc.declare_dram_parameter("y", [NIMG, H, W], F16, isOutput=True)

    with TileContext(nc) as tc, ExitStack() as ctx:
        cpool = ctx.enter_context(tc.tile_pool(name="const", bufs=1))
        c_af = cpool.tile([P, M_EDGE], F32R, tag="caf")
        c_am = cpool.tile([P, M_MID], F32R, tag="cam")
        c_al = cpool.tile([P, M_EDGE], F32R, tag="cal")
        zcol = cpool.tile([P, 1], F32, tag="zcol")
        nc.sync.dma_start(out=c_af[:], in_=a_first[:])
        nc.sync.dma_start(out=c_am[:], in_=a_mid[:])
        nc.sync.dma_start(out=c_al[:], in_=a_last[:])
        nc.vector.memset(zcol[:], 0.0)

        xin = ctx.enter_context(tc.tile_pool(name="xin", bufs=4))
        xin1 = ctx.enter_context(tc.tile_pool(name="xin1", bufs=2))
        psum = ctx.enter_context(tc.tile_pool(name="ps", bufs=2, space="PSUM"))
        padp = ctx.enter_context(tc.tile_pool(name="pad", bufs=3))
        rsp = ctx.enter_context(tc.tile_pool(name="rs", bufs=4))
        outp = ctx.enter_context(tc.tile_pool(name="outp", bufs=3))
        outs = ctx.enter_context(tc.tile_pool(name="outs", bufs=2))

        tiles_by_img = [{} for _ in range(NIMG)]
        # global load units: 8 pair-loads + 1 single per image, prefetched
        # across image boundaries with a sliding window
        load_units = []
        for img in range(NIMG):
            for tp in range(0, NTILE - 1, 2):
                load_units.append((img, tp))
            load_units.append((img, NTILE - 1))
        n_issued = [0]

        def issue_load():
            ui = n_issued[0]
            if ui >= len(load_units):
                return
            n_issued[0] = ui + 1
            img, tp = load_units[ui]
            tiles = tiles_by_img[img]
            eng = nc.sync if ui % 2 == 0 else nc.scalar
            if tp == NTILE - 1:
                buf = xin1.tile([P, W], F32R, tag="xin1", name=f"xin1_{img}")
                eng.dma_start(out=buf[:], in_=x[img, H - P :, :])
                tiles[NTILE - 1] = buf[:]
            else:
                buf = xin.tile([P, 2 * W], F32R, tag="xin", name=f"xin_{img}_{tp}")
                src = (
                    x[img, M_MID * tp : M_MID * tp + P, :]
                    .unsqueeze(1)
                    .broadcast_to([P, 2, W])
                    .copy()
                )
                src.ap[1] = [M_MID * W, 2]
                eng.dma_start(
                    out=buf[:].rearrange("p (c w) -> p c w", c=2), in_=src
                )
                tiles[tp] = buf[:, 0:W]
                tiles[tp + 1] = buf[:, W : 2 * W]

        for _ in range(3):
            issue_load()

        for img in range(NIMG):
            tiles = tiles_by_img[img]
            obuf = None
            for t in range(NTILE):
                # keep the load window 2 units ahead of the consumer
                unit_needed = img * 9 + (t // 2 if t < NTILE - 1 else 8)
                while n_issued[0] <= unit_needed + 2 and n_issued[0] < len(
                    load_units
                ):
                    issue_load()
                m = SIZES[t]
                o_lo = OFFS[t]
                s = psum.tile([P, W], F32, tag="ps")
                c_a = c_af if t == 0 else (c_al if t == NTILE - 1 else c_am)
                for c in range(W // CH):
                    sl = slice(c * CH, (c + 1) * CH)
                    nc.tensor.matmul(
                        s[0:m, sl],
                        c_a[:, 0:m],
                        tiles[t][:, sl],
                        start=True,
                        stop=True,
                    )
                pt = padp.tile([P, PW], F32, tag="pad")
                nc.scalar.copy(pt[0:m, 0:1], zcol[0:m, :])
                nc.scalar.copy(pt[0:m, R + 1 : R + 1 + W], s[0:m, :])
                # reflect pads (reversed order via negative-stride APs)
                nc.scalar.copy(pt[0:m, 1 : R + 1], s[0:m, R:0:-1])
                nc.scalar.copy(pt[0:m, R + 1 + W :], s[0:m, W - 2 : W - 2 - R : -1])
                # running-box-sum recurrence: out[w] = out[w-1] + P[w+9] - P[w],
                # seeded with sum(P[1..8]).
                rs = rsp.tile([P, 1], F32, tag="rs")
                nc.vector.reduce_sum(
                    out=rs[0:m, :], in_=pt[0:m, 1 : 2 * R + 1], axis=mybir.AxisListType.X
                )
                # interior tiles (1,2), (3,4), ... (13,14) pair into one
                # [P, 2W] buffer and store with a single 2MB DMA; tiles
                # 0, 15, 16 store singly.
                paired = False and 1 <= t <= 14
                if paired:
                    if t % 2 == 1:
                        obuf = outp.tile([P, 2 * W], F16, tag="outp")
                    half = (t + 1) % 2  # t odd -> first half, t even -> second
                    o_ap = obuf[0:m, half * W : half * W + W]
                else:
                    o_single = outs.tile([P, W], F16, tag="outs")
                    o_ap = o_single[0:m, :]
                nc.vector.tensor_tensor_scan(
                    out=o_ap,
                    data0=pt[0:m, 2 * R + 1 :],
                    data1=pt[0:m, 0:W],
                    initial=rs[0:m, :],
                    op0=mybir.AluOpType.add,
                    op1=mybir.AluOpType.subtract,
                )
                if paired and t % 2 == 0:
                    dst = (
                        y[img, OFFS[t - 1] : OFFS[t - 1] + M_MID, :]
                        .unsqueeze(1)
                        .broadcast_to([M_MID, 2, W])
                        .copy()
                    )
                    dst.ap[1] = [M_MID * W, 2]
                    seng = nc.scalar if (t // 2) % 2 == 0 else nc.sync
                    seng.dma_start(
                        out=dst,
                        in_=obuf[0:M_MID, :].rearrange("p (c w) -> p c w", c=2),
                    )
                elif not paired:
                    seng = nc.scalar if t % 2 == 0 else nc.sync
                    seng.dma_start(out=y[img, o_lo : o_lo + m, :], in_=o_ap)
    nc.finalize()
    return nc


_CACHE = {}


def _get_setup():
    if "nc" not in _CACHE:
        _CACHE["nc"] = _build_nc()
        _CACHE["blocks"] = _band_blocks()
    return _CACHE["nc"], _CACHE["blocks"]


def kernel(x, r):
    r = int(np.asarray(r))
    assert r == R, f"kernel hardcoded for r={R}, got {r}"
    x = np.asarray(x)
    assert x.shape == (8, 3, H, W) and x.dtype == np.float32, (x.shape, x.dtype)

    nc, (a_first, a_mid, a_last) = _get_setup()
    consts = {"a_first": a_first, "a_mid": a_mid, "a_last": a_last}
    in_maps = [
        {"x": np.ascontiguousarray(x[core]), **consts} for core in range(NCORES)
    ]
    res = run_bass_kernel_spmd(nc, in_maps, core_ids=list(range(NCORES)))
    out = np.stack([res.results[i]["y"] for i in range(NCORES)], axis=0)
    return out.reshape(8, 3, H, W).astype(np.float32)


def _in_maps(x):
    """in_maps for run_bass_kernel_spmd (used by the test harness)."""
    _, (a_first, a_mid, a_last) = _get_setup()
    consts = {"a_first": a_first, "a_mid": a_mid, "a_last": a_last}
    return [
        {"x": np.ascontiguousarray(x[core]), **consts} for core in range(NCORES)
    ]


if __name__ == "__main__":
    rng = np.random.default_rng(0)
    x = rng.standard_normal((8, 3, H, W), dtype=np.float32)
    y = kernel(x, 4)
    print("ran:", y.shape, y.dtype)


# revision 20
# speedup vs baseline: 1.1856x; 1.1856x over previous
"""BoxFilter (9x9 mean filter, reflect padding) Trainium2 Bass kernel.

Input x: [8, 3, 2048, 2048] f32, r=4. Output same shape.

Strategy (per NeuronCore; pure data parallel, batch b -> core b):
  - H-axis box sum via TensorEngine: banded-matrix matmuls (float32r) with
    reflection folded into edge band matrices and the 1/81 scale folded into
    the coefficients. Per 128-row output tile, accumulate contributions from
    the previous tile's last 4 rows, the current tile, and the next tile's
    first 4 rows into PSUM.
  - W-axis box sum via VectorEngine: reflect-pad the H-summed tile into a
    [128, 2057] SBUF tile (ScalarEngine copies from PSUM), run a prefix-sum
    along the free dim (tensor_tensor_scan), then one shifted subtract:
    out[w] = C[w+9] - C[w].
"""

import os
import sys

import numpy as np

for _p in ("/opt/trn_rl_repo", "/opt/pypackages"):
    if os.path.isdir(_p) and _p not in sys.path:
        sys.path.append(_p)

from contextlib import ExitStack

import concourse.bacc as bacc
import concourse.mybir as mybir
from concourse.tile import TileContext
from concourse.bass_utils import run_bass_kernel_spmd

R = 4
NORM = (2 * R + 1) ** 2  # 81
H = W = 2048
P = 128
NT = H // P  # 16 row tiles per image
NIMG = 3  # images per core (batch b -> core b, 3 channels each)
NCORES = 8
CH = 512  # psum chunk (one bank of f32)
F32 = mybir.dt.float32
F32R = mybir.dt.float32r
F16 = mybir.dt.float16
PW = W + 2 * R + 1  # padded width 2057


M_EDGE = P - R  # 124-row edge tiles
M_MID = P - 2 * R  # 120-row interior tiles
SIZES = [M_EDGE] + [M_MID] * 15 + [M_EDGE]  # 17 output tiles per image
OFFS = [0]
for _m in SIZES:
    OFFS.append(OFFS[-1] + _m)
assert OFFS[-1] == H
NTILE = len(SIZES)


def _band_blocks():
    """Banded H-axis operator blocks (lhsT, [K=128, M]) for the 17-tile scheme.

    Output tile t covers rows [OFFS[t], OFFS[t]+SIZES[t]); its input tile is
    the 128 rows [120t, 120t+128). Reflection at the image edges is folded
    into a_first / a_last; one K=128 matmul per output tile per psum chunk.
    """
    hop = np.zeros((H, H), np.float32)
    inv = np.float32(1.0) / np.float32(NORM)
    for i in range(H):
        for dh in range(-R, R + 1):
            g = i + dh
            if g < 0:
                g = -g
            elif g > H - 1:
                g = 2 * (H - 1) - g
            hop[i, g] += inv
    a_first = np.ascontiguousarray(hop[0:M_EDGE, 0:P].T)
    a_mid = np.ascontiguousarray(hop[M_EDGE : M_EDGE + M_MID, M_MID : M_MID + P].T)
    a_last = np.ascontiguousarray(hop[H - M_EDGE :, H - P :].T)
    return a_first, a_mid, a_last


def _build_nc():
    nc = bacc.Bacc("TRN2", target_bir_lowering=False, debug=False, num_devices=1)
    x = nc.declare_dram_parameter("x", [NIMG, H, W], F32R, isOutput=False)
    a_first = nc.declare_dram_parameter("a_first", [P, M_EDGE], F32R, isOutput=False)
    a_mid = nc.declare_dram_parameter("a_mid", [P, M_MID], F32R, isOutput=False)
    a_last = nc.declare_dram_parameter("a_last", [P, M_EDGE], F32R, isOutput=False)
    y = nc.declare_dram_parameter("y", [NIMG, H, W], F16, isOutput=True)

    with TileContext(nc) as tc, ExitStack() as ctx:
        cpool = ctx.enter_context(tc.tile_pool(name="const", bufs=1))
        c_af = cpool.tile([P, M_EDGE], F32R, tag="caf")
        c_am = cpool.tile([P, M_MID], F32R, tag="cam")
        c_al = cpool.tile([P, M_EDGE], F32R, tag="cal")
        zcol = cpool.tile([P, 1], F32, tag="zcol")
        nc.sync.dma_start(out=c_af[:], in_=a_first[:])
        nc.sync.dma_start(out=c_am[:], in_=a_mid[:])
        nc.sync.dma_start(out=c_al[:], in_=a_last[:])
        nc.vector.memset(zcol[:], 0.0)

        xin = ctx.enter_context(tc.tile_pool(name="xin", bufs=4))
        xin1 = ctx.enter_context(tc.tile_pool(name="xin1", bufs=2))
        psum = ctx.enter_context(tc.tile_pool(name="ps", bufs=2, space="PSUM"))
        padp = ctx.enter_context(tc.tile_pool(name="pad", bufs=3))
        rsp = ctx.enter_context(tc.tile_pool(name="rs", bufs=4))
        outp = ctx.enter_context(tc.tile_pool(name="outp", bufs=3))
        outs = ctx.enter_context(tc.tile_pool(name="outs", bufs=8))

        tiles_by_img = [{} for _ in range(NIMG)]
        # global load units: 8 pair-loads + 1 single per image, prefetched
        # across image boundaries with a sliding window
        load_units = []
        for img in range(NIMG):
            for tp in range(0, NTILE - 1, 2):
                load_units.append((img, tp))
            load_units.append((img, NTILE - 1))
        n_issued = [0]

        def issue_load():
            ui = n_issued[0]
            if ui >= len(load_units):
                return
            n_issued[0] = ui + 1
            img, tp = load_units[ui]
            tiles = tiles_by_img[img]
            eng = nc.sync if ui % 2 == 0 else nc.scalar
            if tp == NTILE - 1:
                buf = xin1.tile([P, W], F32R, tag="xin1", name=f"xin1_{img}")
                eng.dma_start(out=buf[:], in_=x[img, H - P :, :])
                tiles[NTILE - 1] = buf[:]
            else:
                buf = xin.tile([P, 2 * W], F32R, tag="xin", name=f"xin_{img}_{tp}")
                src = (
                    x[img, M_MID * tp : M_MID * tp + P, :]
                    .unsqueeze(1)
                    .broadcast_to([P, 2, W])
                    .copy()
                )
                src.ap[1] = [M_MID * W, 2]
                eng.dma_start(
                    out=buf[:].rearrange("p (c w) -> p c w", c=2), in_=src
                )
                tiles[tp] = buf[:, 0:W]
                tiles[tp + 1] = buf[:, W : 2 * W]

        for _ in range(3):
            issue_load()

        for img in range(NIMG):
            tiles = tiles_by_img[img]
            obuf = None
            for t in range(NTILE):
                # keep the load window 2 units ahead of the consumer
                unit_needed = img * 9 + (t // 2 if t < NTILE - 1 else 8)
                while n_issued[0] <= unit_needed + 2 and n_issued[0] < len(
                    load_units
                ):
                    issue_load()
                m = SIZES[t]
                o_lo = OFFS[t]
                s = psum.tile([P, W], F32, tag="ps")
                c_a = c_af if t == 0 else (c_al if t == NTILE - 1 else c_am)
                for c in range(W // CH):
                    sl = slice(c * CH, (c + 1) * CH)
                    nc.tensor.matmul(
                        s[0:m, sl],
                        c_a[:, 0:m],
                        tiles[t][:, sl],
                        start=True,
                        stop=True,
                    )
                pt = padp.tile([P, PW], F32, tag="pad")
                nc.scalar.copy(pt[0:m, 0:1], zcol[0:m, :])
                nc.scalar.copy(pt[0:m, R + 1 : R + 1 + W], s[0:m, :])
                # reflect pads (reversed order via negative-stride APs)
                nc.scalar.copy(pt[0:m, 1 : R + 1], s[0:m, R:0:-1])
                nc.scalar.copy(pt[0:m, R + 1 + W :], s[0:m, W - 2 : W - 2 - R : -1])
                # running-box-sum recurrence: out[w] = out[w-1] + P[w+9] - P[w],
                # seeded with sum(P[1..8]).
                rs = rsp.tile([P, 1], F32, tag="rs")
                nc.vector.reduce_sum(
                    out=rs[0:m, :], in_=pt[0:m, 1 : 2 * R + 1], axis=mybir.AxisListType.X
                )
                # interior tiles (1,2), (3,4), ... (13,14) pair into one
                # [P, 2W] buffer and store with a single 2MB DMA; tiles
                # 0, 15, 16 store singly.
                paired = False and 1 <= t <= 14
                if paired:
                    if t % 2 == 1:
                        obuf = outp.tile([P, 2 * W], F16, tag="outp")
                    half = (t + 1) % 2  # t odd -> first half, t even -> second
                    o_ap = obuf[0:m, half * W : half * W + W]
                else:
                    o_single = outs.tile([P, W], F16, tag="outs")
                    o_ap = o_single[0:m, :]
                nc.vector.tensor_tensor_scan(
                    out=o_ap,
                    data0=pt[0:m, 2 * R + 1 :],
                    data1=pt[0:m, 0:W],
                    initial=rs[0:m, :],
                    op0=mybir.AluOpType.add,
                    op1=mybir.AluOpType.subtract,
                )
                if paired and t % 2 == 0:
                    dst = (
                        y[img, OFFS[t - 1] : OFFS[t - 1] + M_MID, :]
                        .unsqueeze(1)
                        .broadcast_to([M_MID, 2, W])
                        .copy()
                    )
                    dst.ap[1] = [M_MID * W, 2]
                    seng = nc.scalar if (t // 2) % 2 == 0 else nc.sync
                    seng.dma_start(
                        out=dst,
                        in_=obuf[0:M_MID, :].rearrange("p (c w) -> p c w", c=2),
                    )
                elif not paired:
                    seng = nc.scalar if t % 2 == 0 else nc.sync
                    seng.dma_start(out=y[img, o_lo : o_lo + m, :], in_=o_ap)
    nc.finalize()
    return nc


_CACHE = {}


def _get_setup():
    if "nc" not in _CACHE:
        _CACHE["nc"] = _build_nc()
        _CACHE["blocks"] = _band_blocks()
    return _CACHE["nc"], _CACHE["blocks"]


def kernel(x, r):
    r = int(np.asarray(r))
    assert r == R, f"kernel hardcoded for r={R}, got {r}"
    x = np.asarray(x)
    assert x.shape == (8, 3, H, W) and x.dtype == np.float32, (x.shape, x.dtype)

    nc, (a_first, a_mid, a_last) = _get_setup()
    consts = {"a_first": a_first, "a_mid": a_mid, "a_last": a_last}
    in_maps = [
        {"x": np.ascontiguousarray(x[core]), **consts} for core in range(NCORES)
    ]
    res = run_bass_kernel_spmd(nc, in_maps, core_ids=list(range(NCORES)))
    out = np.stack([res.results[i]["y"] for i in range(NCORES)], axis=0)
    return out.reshape(8, 3, H, W).astype(np.float32)


def _in_maps(x):
    """in_maps for run_bass_kernel_spmd (used by the test harness)."""
    _, (a_first, a_mid, a_last) = _get_setup()
    consts = {"a_first": a_first, "a_mid": a_mid, "a_last": a_last}
    return [
        {"x": np.ascontiguousarray(x[core]), **consts} for core in range(NCORES)
    ]


if __name__ == "__main__":
    rng = np.random.default_rng(0)
    x = rng.standard_normal((8, 3, H, W), dtype=np.float32)
    y = kernel(x, 4)
    print("ran:", y.shape, y.dtype)
